# revision 12
# baseline (speedup 1.0000x reference)
"""TRN2 Bass kernel for nn_Attention_87497073754296.

Computes, for Y [4096, 1024] f32 and W_param [1024, 1024] f32:
    G = Y @ W_param.T ; S = G @ G.T ; A = softmax(S, -1) ; Z = A @ Y
using the identity S = Y @ (W_param.T @ W_param) @ Y.T, so each core only
needs its own row-shard of the queries plus the (replicated) full Y — no
collectives. M = W_param.T @ W_param (symmetric, d x d) is computed once
on the host and replicated.

Sharding: rows of Y (queries), 512 per core across 8 cores.

Per core:
    Ht  = (Yq @ M).T        fp8e4m3 DoubleRow matmuls (scores tolerate fp8)
    S   = Ht.T @ Y.T        fp8e4m3 DoubleRow, evicted fp16
    P   = exp(S - rowmax)   ACT exp, accum_out row sums
    Pt  = P.T               PE transposes (fp16)
    Z   = (Pt.T @ (Yh + Ym)) * (1/rowsum)
where Yh = fp16(Y), Ym = fp16(Y - Yh). fp16 has 11 mantissa bits, so
Yh + Ym carries >= 23 bits: fp16 x fp16 products are exact in fp32, the
PSUM accumulation reconstructs A @ Y to within 1 fp32 ulp, and both
passes run at 1 cycle/row like bf16.

Scheduling keeps the PE stream dense so the HAM clock gate never
re-throttles:
  - q-tile t's softmax (exp on ACT, transposes on PE, copies/maxes on
    DVE) is interleaved chunk-by-chunk into q-tile t+1's score matmuls;
    score evictions alternate ACT/DVE so no single engine paces the PE;
  - the last q-tile's softmax overlaps the start of the A@Y matmuls for
    q-tiles 0..2 via three PSUM pools (6-bank Z pool + 2-bank transpose
    pool, then the 4th accumulator takes the transpose pool's banks).
"""
import numpy as np
import ml_dtypes

import concourse.bass as bass
import concourse.mybir as mybir
import concourse.tile as tile
from concourse import bacc
from concourse.bass_utils import run_bass_kernel_spmd
from concourse.masks import make_identity

F32 = mybir.dt.float32
FP16 = mybir.dt.float16
FP8 = mybir.dt.float8e4
DR = mybir.MatmulPerfMode.DoubleRow
AF = mybir.ActivationFunctionType
AX = mybir.AxisListType
OP = mybir.AluOpType

N, D = 4096, 1024
CORES = 8
QSH = N // CORES          # 512 queries per core
P = 128                   # partitions
DT = D // P               # 8 d-subtiles
QT = QSH // P             # 4 q-tiles per core
JC = N // 512             # 8 j-chunks of 512 for scores
JT = N // P               # 32 j-tiles of 128 for A@Y

_CACHED = {}


def _build():
    nc = bacc.Bacc("TRN2", target_bir_lowering=False, debug=False,
                   num_devices=CORES)
    M8 = nc.declare_dram_parameter("M8", [D, D], FP8, isOutput=False)
    Yqt8 = nc.declare_dram_parameter("Yqt8", [D, QSH], FP8, isOutput=False)
    Yt8 = nc.declare_dram_parameter("Yt8", [D, N], FP8, isOutput=False)
    Yh = nc.declare_dram_parameter("Yh", [N, D], FP16, isOutput=False)
    Ym = nc.declare_dram_parameter("Ym", [N, D], FP16, isOutput=False)
    Z = nc.declare_dram_parameter("Z", [QSH, D], F32, isOutput=True)

    with tile.TileContext(nc) as tc:
        with (
            tc.tile_pool(name="const", bufs=1) as const,
            tc.tile_pool(name="stat", bufs=1) as stat,
            tc.tile_pool(name="htpool", bufs=1) as htpool,
            tc.tile_pool(name="spool", bufs=1) as spool,
            tc.tile_pool(name="epool", bufs=2) as epool,
            tc.tile_pool(name="ptpool", bufs=1) as ptpool,
            tc.tile_pool(name="mpool", bufs=1) as mpool,
            tc.tile_pool(name="yqpool", bufs=1) as yqpool,
            tc.tile_pool(name="ytpool", bufs=1) as ytpool,
            tc.tile_pool(name="yzpool", bufs=4) as yzpool,
            tc.tile_pool(name="zopool", bufs=2) as zopool,
        ):
            # warmup tile initialized on DVE (no gpsimd ucode-load delay)
            wtile = const.tile([P, P], FP16, name="wtile")
            nc.vector.memset(wtile[:], 1.0)
            with tc.tile_pool(name="warm", bufs=1, space="PSUM") as warm:
                wp = warm.tile([P, P], FP16, name="wp")
                for _ in range(230):
                    nc.tensor.transpose(wp[:], wtile[:], wtile[:])

            ident = const.tile([P, P], FP16, name="ident")
            make_identity(nc, ident[:])

            # ---- resident loads (separate queues) ----
            m_sb = mpool.tile([P, DT, D], FP8, name="m_sb")
            yq_sb = yqpool.tile([P, DT, QSH], FP8, name="yq_sb")
            nc.sync.dma_start(m_sb[:], M8.rearrange("(di p) f -> p di f", p=P))
            nc.sync.dma_start(
                yq_sb[:], Yqt8.rearrange("(di p) f -> p di f", p=P))
            yt_sb = ytpool.tile([P, DT, N], FP8, name="yt_sb")
            nc.gpsimd.dma_start(
                yt_sb[:], Yt8.rearrange("(di p) f -> p di f", p=P))

            # stats: negmax/recip [P, QT], chunk maxes, half sums, rowsum
            st = stat.tile([P, 64], F32, name="st")
            negmax = st[:, 0:QT]
            recip = st[:, QT:2 * QT]
            mx8 = st[:, 8:8 + QT * JC]
            rowsum = st[:, 40:44]
            esum = st[:, 44:44 + 2 * QT]

            ht_sb = htpool.tile([P, DT, QSH], FP8, name="ht_sb")
            s_sb = [
                spool.tile([P, N], FP16, name=f"s_sb{t}", tag=f"s{t}")
                for t in range(QT)
            ]
            pt_sb = [
                ptpool.tile([P, N], FP16, name=f"pt_sb{t}", tag=f"pt{t}")
                for t in range(QT)
            ]
            e_tiles = {}

            def emit_exp_half(t, h, pool):
                e = pool.tile([P, N // 2], FP16, name="e_sb", tag="e")
                e_tiles[(t, h)] = e
                nc.scalar.activation(
                    e[:], s_sb[t][:, h * (N // 2):(h + 1) * (N // 2)],
                    AF.Exp, bias=negmax[:, t:t + 1], scale=1.0,
                    accum_out=esum[:, 2 * t + h:2 * t + h + 1],
                )

            def emit_T_chunk(t, c, pool):
                """Transpose chunk c (512 cols) of E(t) into pt_sb[t]."""
                e = e_tiles[(t, c // 4)]
                off = (c % 4) * 512
                pp = pool.tile([P, 512], FP16, name="pp", tag="pt")
                for k in range(4):
                    nc.tensor.transpose(
                        pp[:, k * P:(k + 1) * P],
                        e[:, off + k * P: off + (k + 1) * P],
                        ident[:],
                    )
                nc.vector.tensor_copy(
                    pt_sb[t][:, c * 512:(c + 1) * 512], pp[:])

            def emit_finish_rowsum(t):
                nc.vector.tensor_reduce(
                    rowsum[:, t:t + 1], esum[:, 2 * t:2 * t + 2],
                    axis=AX.X, op=OP.add,
                )
                nc.vector.reciprocal(recip[:, t:t + 1], rowsum[:, t:t + 1])

            def emit_negmax(t):
                nc.vector.tensor_reduce(
                    negmax[:, t:t + 1], mx8[:, t * JC:(t + 1) * JC],
                    axis=AX.X, op=OP.max, negate=True,
                )

            with tc.tile_pool(name="ps", bufs=2, space="PSUM") as ps:
                # ---- H: Ht[do, q] = sum_di M[di,do].T @ Yqt[di, q] ----
                for dt_ in range(DT):
                    hp = ps.tile([P, QSH], F32, name="hp", tag="s")
                    for s in range(DT // 2):
                        nc.tensor.matmul(
                            hp[:],
                            m_sb[:, 2 * s:2 * s + 2, dt_ * P:(dt_ + 1) * P],
                            yq_sb[:, 2 * s:2 * s + 2, :],
                            start=(s == 0), stop=(s == DT // 2 - 1),
                            perf_mode=DR,
                        )
                    nc.scalar.copy(ht_sb[:, dt_, :], hp[:])

                # ---- S (t-outer), softmax of t-1 interleaved per chunk ----
                for t in range(QT):
                    if t >= 1:
                        emit_negmax(t - 1)
                        emit_exp_half(t - 1, 0, epool)
                    for jc in range(JC):
                        sp = ps.tile([P, 512], F32, name="sp", tag="s")
                        for s in range(DT // 2):
                            nc.tensor.matmul(
                                sp[:],
                                ht_sb[:, 2 * s:2 * s + 2, t * P:(t + 1) * P],
                                yt_sb[:, 2 * s:2 * s + 2,
                                      jc * 512:(jc + 1) * 512],
                                start=(s == 0), stop=(s == DT // 2 - 1),
                                perf_mode=DR,
                            )
                        # alternate eviction engine so neither ACT nor DVE
                        # paces the PE
                        dst = s_sb[t][:, jc * 512:(jc + 1) * 512]
                        if jc % 2 == 1:
                            nc.vector.tensor_copy(dst, sp[:])
                        else:
                            nc.scalar.copy(dst, sp[:])
                        nc.vector.tensor_reduce(
                            mx8[:, t * JC + jc: t * JC + jc + 1], dst,
                            axis=AX.X, op=OP.max,
                        )
                        if t >= 1:
                            if jc == 3:
                                emit_exp_half(t - 1, 1, epool)
                            if jc >= 1:
                                emit_T_chunk(t - 1, jc - 1, ps)
                    if t >= 1:
                        emit_T_chunk(t - 1, JC - 1, ps)
                        emit_finish_rowsum(t - 1)

            # ---- boundary: softmax(3) overlapped with Z for t=0..2 ----
            tl = QT - 1
            with tc.tile_pool(name="psZ", bufs=3, space="PSUM") as psZ:
                zp = {
                    t: psZ.tile([P, D], F32, name=f"zp{t}", tag="z")
                    for t in range(3)
                }

                def z_mms(jt, yz, ts):
                    for t in ts:
                        for dc in range(2):
                            for s in range(2):
                                nc.tensor.matmul(
                                    zp[t][:, dc * 512:(dc + 1) * 512],
                                    pt_sb[t][:, jt * P:(jt + 1) * P],
                                    yz[:, s, dc * 512:dc * 512 + 512],
                                    start=(jt == 0 and s == 0),
                                    stop=(jt == JT - 1 and s == 1),
                                )

                def yz_load(jt):
                    yz = yzpool.tile([P, 2, D], FP16, name="yz")
                    nc.sync.dma_start(yz[:, 0, :], Yh[jt * P:(jt + 1) * P, :])
                    nc.sync.dma_start(yz[:, 1, :], Ym[jt * P:(jt + 1) * P, :])
                    return yz

                yzs = {}
                with tc.tile_pool(name="pp3", bufs=2, space="PSUM") as pp3:
                    emit_negmax(tl)
                    emit_exp_half(tl, 0, epool)
                    for jt in range(4):
                        yzs[jt] = yz_load(jt)
                        z_mms(jt, yzs[jt], (0, 1, 2))
                        if jt == 1:
                            emit_exp_half(tl, 1, epool)
                        for c in (2 * jt, 2 * jt + 1):
                            emit_T_chunk(tl, c, pp3)
                    emit_finish_rowsum(tl)

                # 4th accumulator takes the banks freed by pp3
                with tc.tile_pool(name="psZ2", bufs=1, space="PSUM") as psZ2:
                    zp[3] = psZ2.tile([P, D], F32, name="zp3", tag="z3")
                    for jt in range(4):
                        z_mms(jt, yzs[jt], (3,))
                    for jt in range(4, JT):
                        yz = yz_load(jt)
                        z_mms(jt, yz, (0, 1, 2, 3))
                    for t in range(QT):
                        zo = zopool.tile([P, D], F32, name="zo", tag="zo")
                        nc.scalar.activation(
                            zo[:], zp[t][:], AF.Copy, bias=0.0,
                            scale=recip[:, t:t + 1],
                        )
                        nc.sync.dma_start(Z[t * P:(t + 1) * P, :], zo[:])

    nc.finalize()
    return nc


def _prep_inputs(Y: np.ndarray, W_param: np.ndarray):
    Y = np.ascontiguousarray(Y, dtype=np.float32)
    W = np.ascontiguousarray(W_param, dtype=np.float32)
    M = (W.T @ W).astype(np.float32)
    Yh = Y.astype(np.float16)
    Ym = (Y - Yh.astype(np.float32)).astype(np.float16)
    M8 = M.astype(ml_dtypes.float8_e4m3)
    Yt8 = np.ascontiguousarray(Y.T).astype(ml_dtypes.float8_e4m3)
    in_maps = []
    for c in range(CORES):
        in_maps.append({
            "M8": M8,
            "Yqt8": np.ascontiguousarray(Yt8[:, c * QSH:(c + 1) * QSH]),
            "Yt8": Yt8,
            "Yh": Yh,
            "Ym": Ym,
        })
    return in_maps


def _run(inputs: dict, trace: bool = False):
    Y = np.asarray(inputs["Y"])
    W = np.asarray(inputs["W_param"])
    assert Y.shape == (N, D) and W.shape == (D, D)
    if "nc" not in _CACHED:
        _CACHED["nc"] = _build()
    nc = _CACHED["nc"]
    in_maps = _prep_inputs(Y, W)
    res = run_bass_kernel_spmd(nc, in_maps, list(range(CORES)), trace=trace)
    out = np.concatenate(
        [res.results[c]["Z"] for c in range(CORES)], axis=0
    ).astype(np.float32)
    return out, res


def kernel(Y: np.ndarray, W_param: np.ndarray) -> np.ndarray:
    out, _ = _run({"Y": Y, "W_param": W_param})
    return out


# revision 13
# speedup vs baseline: 1.0143x; 1.0143x over previous
"""TRN2 Bass kernel for nn_Attention_87497073754296.

Computes, for Y [4096, 1024] f32 and W_param [1024, 1024] f32:
    G = Y @ W_param.T ; S = G @ G.T ; A = softmax(S, -1) ; Z = A @ Y
using the identity S = Y @ (W_param.T @ W_param) @ Y.T, so each core only
needs its own row-shard of the queries plus the (replicated) full Y — no
collectives. M = W_param.T @ W_param (symmetric, d x d) is computed once
on the host and replicated.

Sharding: rows of Y (queries), 512 per core across 8 cores.

Per core:
    Ht  = (Yq @ M).T        fp8e4m3 DoubleRow matmuls (scores tolerate fp8)
    S   = Ht.T @ Y.T        fp8e4m3 DoubleRow, evicted fp16
    P   = exp(S - rowmax)   ACT exp, accum_out row sums
    Pt  = P.T               PE transposes (fp16)
    Z   = (Pt.T @ (Yh + Ym)) * (1/rowsum)
where Yh = fp16(Y), Ym = fp16(Y - Yh). fp16 has 11 mantissa bits, so
Yh + Ym carries >= 23 bits: fp16 x fp16 products are exact in fp32, the
PSUM accumulation reconstructs A @ Y to within 1 fp32 ulp, and both
passes run at 1 cycle/row like bf16.

Scheduling keeps the PE stream dense so the HAM clock gate never
re-throttles:
  - q-tile t's softmax (exp on ACT, transposes on PE, copies/maxes on
    DVE) is interleaved chunk-by-chunk into q-tile t+1's score matmuls;
    score evictions alternate ACT/DVE so no single engine paces the PE;
  - the last q-tile's softmax overlaps the start of the A@Y matmuls for
    q-tiles 0..2 via three PSUM pools (6-bank Z pool + 2-bank transpose
    pool, then the 4th accumulator takes the transpose pool's banks).
"""
import numpy as np
import ml_dtypes

import concourse.bass as bass
import concourse.mybir as mybir
import concourse.tile as tile
from concourse import bacc
from concourse.bass_utils import run_bass_kernel_spmd
from concourse.masks import make_identity

F32 = mybir.dt.float32
FP16 = mybir.dt.float16
FP8 = mybir.dt.float8e4
DR = mybir.MatmulPerfMode.DoubleRow
AF = mybir.ActivationFunctionType
AX = mybir.AxisListType
OP = mybir.AluOpType

N, D = 4096, 1024
CORES = 8
QSH = N // CORES          # 512 queries per core
P = 128                   # partitions
DT = D // P               # 8 d-subtiles
QT = QSH // P             # 4 q-tiles per core
JC = N // 512             # 8 j-chunks of 512 for scores
JT = N // P               # 32 j-tiles of 128 for A@Y

_CACHED = {}


def _build():
    nc = bacc.Bacc("TRN2", target_bir_lowering=False, debug=False,
                   num_devices=CORES)
    M8 = nc.declare_dram_parameter("M8", [D, D], FP8, isOutput=False)
    Yqt8 = nc.declare_dram_parameter("Yqt8", [D, QSH], FP8, isOutput=False)
    Yt8 = nc.declare_dram_parameter("Yt8", [D, N], FP8, isOutput=False)
    Yh = nc.declare_dram_parameter("Yh", [N, D], FP16, isOutput=False)
    Ym = nc.declare_dram_parameter("Ym", [N, D], FP16, isOutput=False)
    Z = nc.declare_dram_parameter("Z", [QSH, D], F32, isOutput=True)

    with tile.TileContext(nc) as tc:
        with (
            tc.tile_pool(name="const", bufs=1) as const,
            tc.tile_pool(name="stat", bufs=1) as stat,
            tc.tile_pool(name="htpool", bufs=1) as htpool,
            tc.tile_pool(name="spool", bufs=1) as spool,
            tc.tile_pool(name="epool", bufs=2) as epool,
            tc.tile_pool(name="ptpool", bufs=1) as ptpool,
            tc.tile_pool(name="mpool", bufs=1) as mpool,
            tc.tile_pool(name="yqpool", bufs=1) as yqpool,
            tc.tile_pool(name="ytpool", bufs=1) as ytpool,
            tc.tile_pool(name="yzpool", bufs=4) as yzpool,
            tc.tile_pool(name="zopool", bufs=2) as zopool,
        ):
            # warmup tile initialized on DVE (no gpsimd ucode-load delay)
            wtile = const.tile([P, P], FP16, name="wtile")
            nc.vector.memset(wtile[:], 1.0)
            with tc.tile_pool(name="warm", bufs=1, space="PSUM") as warm:
                wp = warm.tile([P, P], FP16, name="wp")
                for _ in range(230):
                    nc.tensor.transpose(wp[:], wtile[:], wtile[:])

            ident = const.tile([P, P], FP16, name="ident")
            make_identity(nc, ident[:])

            # ---- resident loads (separate queues) ----
            m_sb = mpool.tile([P, DT, D], FP8, name="m_sb")
            yq_sb = yqpool.tile([P, DT, QSH], FP8, name="yq_sb")
            nc.sync.dma_start(m_sb[:], M8.rearrange("(di p) f -> p di f", p=P))
            nc.sync.dma_start(
                yq_sb[:], Yqt8.rearrange("(di p) f -> p di f", p=P))
            yt_sb = ytpool.tile([P, DT, N], FP8, name="yt_sb")
            nc.gpsimd.dma_start(
                yt_sb[:], Yt8.rearrange("(di p) f -> p di f", p=P))

            # stats: negmax/recip [P, QT], chunk maxes, half sums, rowsum
            st = stat.tile([P, 64], F32, name="st")
            negmax = st[:, 0:QT]
            recip = st[:, QT:2 * QT]
            mx8 = st[:, 8:8 + QT * JC]
            rowsum = st[:, 40:44]
            esum = st[:, 44:44 + 2 * QT]

            ht_sb = htpool.tile([P, DT, QSH], FP8, name="ht_sb")
            s_sb = [
                spool.tile([P, N], FP16, name=f"s_sb{t}", tag=f"s{t}")
                for t in range(QT)
            ]
            pt_sb = [
                ptpool.tile([P, N], FP16, name=f"pt_sb{t}", tag=f"pt{t}")
                for t in range(QT)
            ]
            e_tiles = {}

            def emit_exp_half(t, h, pool):
                e = pool.tile([P, N // 2], FP16, name="e_sb", tag="e")
                e_tiles[(t, h)] = e
                nc.scalar.activation(
                    e[:], s_sb[t][:, h * (N // 2):(h + 1) * (N // 2)],
                    AF.Exp, bias=negmax[:, t:t + 1], scale=1.0,
                    accum_out=esum[:, 2 * t + h:2 * t + h + 1],
                )

            def emit_T_chunk(t, c, pool):
                """Transpose chunk c (512 cols) of E(t) into pt_sb[t]."""
                e = e_tiles[(t, c // 4)]
                off = (c % 4) * 512
                pp = pool.tile([P, 512], FP16, name="pp", tag="pt")
                for k in range(4):
                    nc.tensor.transpose(
                        pp[:, k * P:(k + 1) * P],
                        e[:, off + k * P: off + (k + 1) * P],
                        ident[:],
                    )
                nc.vector.tensor_copy(
                    pt_sb[t][:, c * 512:(c + 1) * 512], pp[:])

            def emit_finish_rowsum(t):
                nc.vector.tensor_reduce(
                    rowsum[:, t:t + 1], esum[:, 2 * t:2 * t + 2],
                    axis=AX.X, op=OP.add,
                )
                nc.vector.reciprocal(recip[:, t:t + 1], rowsum[:, t:t + 1])

            def emit_negmax(t):
                nc.vector.tensor_reduce(
                    negmax[:, t:t + 1], mx8[:, t * JC:(t + 1) * JC],
                    axis=AX.X, op=OP.max, negate=True,
                )

            with tc.tile_pool(name="ps", bufs=2, space="PSUM") as ps:
                # ---- H: Ht[do, q] = sum_di M[di,do].T @ Yqt[di, q] ----
                for dt_ in range(DT):
                    hp = ps.tile([P, QSH], F32, name="hp", tag="s")
                    for s in range(DT // 2):
                        nc.tensor.matmul(
                            hp[:],
                            m_sb[:, 2 * s:2 * s + 2, dt_ * P:(dt_ + 1) * P],
                            yq_sb[:, 2 * s:2 * s + 2, :],
                            start=(s == 0), stop=(s == DT // 2 - 1),
                            perf_mode=DR,
                        )
                    nc.scalar.copy(ht_sb[:, dt_, :], hp[:])

                # ---- S (t-outer), softmax of t-1 interleaved per chunk ----
                for t in range(QT):
                    if t >= 1:
                        emit_negmax(t - 1)
                        emit_exp_half(t - 1, 0, epool)
                    for jc in range(JC):
                        sp = ps.tile([P, 512], F32, name="sp", tag="s")
                        for s in range(DT // 2):
                            nc.tensor.matmul(
                                sp[:],
                                ht_sb[:, 2 * s:2 * s + 2, t * P:(t + 1) * P],
                                yt_sb[:, 2 * s:2 * s + 2,
                                      jc * 512:(jc + 1) * 512],
                                start=(s == 0), stop=(s == DT // 2 - 1),
                                perf_mode=DR,
                            )
                        # alternate eviction engine so neither ACT nor DVE
                        # paces the PE
                        dst = s_sb[t][:, jc * 512:(jc + 1) * 512]
                        if jc % 3 == 2:
                            nc.vector.tensor_copy(dst, sp[:])
                        else:
                            nc.scalar.copy(dst, sp[:])
                        nc.vector.tensor_reduce(
                            mx8[:, t * JC + jc: t * JC + jc + 1], dst,
                            axis=AX.X, op=OP.max,
                        )
                        if t >= 1:
                            if jc == 3:
                                emit_exp_half(t - 1, 1, epool)
                            if jc >= 1:
                                emit_T_chunk(t - 1, jc - 1, ps)
                    if t >= 1:
                        emit_T_chunk(t - 1, JC - 1, ps)
                        emit_finish_rowsum(t - 1)

            # ---- boundary: softmax(3) overlapped with Z for t=0..2 ----
            tl = QT - 1
            with tc.tile_pool(name="psZ", bufs=3, space="PSUM") as psZ:
                zp = {
                    t: psZ.tile([P, D], F32, name=f"zp{t}", tag="z")
                    for t in range(3)
                }

                def z_mms(jt, yz, ts):
                    for t in ts:
                        for dc in range(2):
                            for s in range(2):
                                nc.tensor.matmul(
                                    zp[t][:, dc * 512:(dc + 1) * 512],
                                    pt_sb[t][:, jt * P:(jt + 1) * P],
                                    yz[:, s, dc * 512:dc * 512 + 512],
                                    start=(jt == 0 and s == 0),
                                    stop=(jt == JT - 1 and s == 1),
                                )

                def yz_load(jt):
                    yz = yzpool.tile([P, 2, D], FP16, name="yz")
                    nc.sync.dma_start(yz[:, 0, :], Yh[jt * P:(jt + 1) * P, :])
                    nc.sync.dma_start(yz[:, 1, :], Ym[jt * P:(jt + 1) * P, :])
                    return yz

                yzs = {}
                with tc.tile_pool(name="pp3", bufs=2, space="PSUM") as pp3:
                    emit_negmax(tl)
                    emit_exp_half(tl, 0, epool)
                    for jt in range(4):
                        yzs[jt] = yz_load(jt)
                        z_mms(jt, yzs[jt], (0, 1, 2))
                        if jt == 1:
                            emit_exp_half(tl, 1, epool)
                        for c in (2 * jt, 2 * jt + 1):
                            emit_T_chunk(tl, c, pp3)
                    emit_finish_rowsum(tl)

                # 4th accumulator takes the banks freed by pp3
                with tc.tile_pool(name="psZ2", bufs=1, space="PSUM") as psZ2:
                    zp[3] = psZ2.tile([P, D], F32, name="zp3", tag="z3")
                    for jt in range(4):
                        z_mms(jt, yzs[jt], (3,))
                    for jt in range(4, JT):
                        yz = yz_load(jt)
                        z_mms(jt, yz, (0, 1, 2, 3))
                    for t in range(QT):
                        zo = zopool.tile([P, D], F32, name="zo", tag="zo")
                        nc.scalar.activation(
                            zo[:], zp[t][:], AF.Copy, bias=0.0,
                            scale=recip[:, t:t + 1],
                        )
                        nc.sync.dma_start(Z[t * P:(t + 1) * P, :], zo[:])

    nc.finalize()
    return nc


def _prep_inputs(Y: np.ndarray, W_param: np.ndarray):
    Y = np.ascontiguousarray(Y, dtype=np.float32)
    W = np.ascontiguousarray(W_param, dtype=np.float32)
    M = (W.T @ W).astype(np.float32)
    Yh = Y.astype(np.float16)
    Ym = (Y - Yh.astype(np.float32)).astype(np.float16)
    M8 = M.astype(ml_dtypes.float8_e4m3)
    Yt8 = np.ascontiguousarray(Y.T).astype(ml_dtypes.float8_e4m3)
    in_maps = []
    for c in range(CORES):
        in_maps.append({
            "M8": M8,
            "Yqt8": np.ascontiguousarray(Yt8[:, c * QSH:(c + 1) * QSH]),
            "Yt8": Yt8,
            "Yh": Yh,
            "Ym": Ym,
        })
    return in_maps


def _run(inputs: dict, trace: bool = False):
    Y = np.asarray(inputs["Y"])
    W = np.asarray(inputs["W_param"])
    assert Y.shape == (N, D) and W.shape == (D, D)
    if "nc" not in _CACHED:
        _CACHED["nc"] = _build()
    nc = _CACHED["nc"]
    in_maps = _prep_inputs(Y, W)
    res = run_bass_kernel_spmd(nc, in_maps, list(range(CORES)), trace=trace)
    out = np.concatenate(
        [res.results[c]["Z"] for c in range(CORES)], axis=0
    ).astype(np.float32)
    return out, res


def kernel(Y: np.ndarray, W_param: np.ndarray) -> np.ndarray:
    out, _ = _run({"Y": Y, "W_param": W_param})
    return out


# revision 14
# speedup vs baseline: 1.0207x; 1.0063x over previous
"""TRN2 Bass kernel for nn_Attention_87497073754296.

Computes, for Y [4096, 1024] f32 and W_param [1024, 1024] f32:
    G = Y @ W_param.T ; S = G @ G.T ; A = softmax(S, -1) ; Z = A @ Y
using the identity S = Y @ (W_param.T @ W_param) @ Y.T, so each core only
needs its own row-shard of the queries plus the (replicated) full Y — no
collectives. M = W_param.T @ W_param (symmetric, d x d) is computed once
on the host and replicated.

Sharding: rows of Y (queries), 512 per core across 8 cores.

Per core:
    Ht  = (Yq @ M).T        fp8e4m3 DoubleRow matmuls (scores tolerate fp8)
    S   = Ht.T @ Y.T        fp8e4m3 DoubleRow, evicted fp16
    P   = exp(S - rowmax)   ACT exp, accum_out row sums
    Pt  = P.T               PE transposes (fp16)
    Z   = (Pt.T @ (Yh + Ym)) * (1/rowsum)
where Yh = fp16(Y), Ym = fp16(Y - Yh). fp16 has 11 mantissa bits, so
Yh + Ym carries >= 23 bits: fp16 x fp16 products are exact in fp32, the
PSUM accumulation reconstructs A @ Y to within 1 fp32 ulp, and both
passes run at 1 cycle/row like bf16.

Scheduling keeps the PE stream dense so the HAM clock gate never
re-throttles:
  - q-tile t's softmax (exp on ACT, transposes on PE, copies/maxes on
    DVE) is interleaved chunk-by-chunk into q-tile t+1's score matmuls;
    score evictions alternate ACT/DVE so no single engine paces the PE;
  - the last q-tile's softmax overlaps the start of the A@Y matmuls for
    q-tiles 0..2 via three PSUM pools (6-bank Z pool + 2-bank transpose
    pool, then the 4th accumulator takes the transpose pool's banks).
"""
import numpy as np
import ml_dtypes

import concourse.bass as bass
import concourse.mybir as mybir
import concourse.tile as tile
from concourse import bacc
from concourse.bass_utils import run_bass_kernel_spmd
from concourse.masks import make_identity

F32 = mybir.dt.float32
FP16 = mybir.dt.float16
FP8 = mybir.dt.float8e4
DR = mybir.MatmulPerfMode.DoubleRow
AF = mybir.ActivationFunctionType
AX = mybir.AxisListType
OP = mybir.AluOpType

N, D = 4096, 1024
CORES = 8
QSH = N // CORES          # 512 queries per core
P = 128                   # partitions
DT = D // P               # 8 d-subtiles
QT = QSH // P             # 4 q-tiles per core
JC = N // 512             # 8 j-chunks of 512 for scores
JT = N // P               # 32 j-tiles of 128 for A@Y

_CACHED = {}


def _build():
    nc = bacc.Bacc("TRN2", target_bir_lowering=False, debug=False,
                   num_devices=CORES)
    M8 = nc.declare_dram_parameter("M8", [D, D], FP8, isOutput=False)
    Yqt8 = nc.declare_dram_parameter("Yqt8", [D, QSH], FP8, isOutput=False)
    Yt8 = nc.declare_dram_parameter("Yt8", [D, N], FP8, isOutput=False)
    Yh = nc.declare_dram_parameter("Yh", [N, D], FP16, isOutput=False)
    Ym = nc.declare_dram_parameter("Ym", [N, D], FP16, isOutput=False)
    Z = nc.declare_dram_parameter("Z", [QSH, D], F32, isOutput=True)

    with tile.TileContext(nc) as tc:
        with (
            tc.tile_pool(name="const", bufs=1) as const,
            tc.tile_pool(name="stat", bufs=1) as stat,
            tc.tile_pool(name="htpool", bufs=1) as htpool,
            tc.tile_pool(name="spool", bufs=1) as spool,
            tc.tile_pool(name="epool", bufs=2) as epool,
            tc.tile_pool(name="ptpool", bufs=1) as ptpool,
            tc.tile_pool(name="mpool", bufs=1) as mpool,
            tc.tile_pool(name="yqpool", bufs=1) as yqpool,
            tc.tile_pool(name="ytpool", bufs=1) as ytpool,
            tc.tile_pool(name="yzpool", bufs=4) as yzpool,
            tc.tile_pool(name="zopool", bufs=2) as zopool,
        ):
            # warmup tile initialized on DVE (no gpsimd ucode-load delay)
            wtile = const.tile([P, P], FP16, name="wtile")
            nc.vector.memset(wtile[:], 1.0)
            with tc.tile_pool(name="warm", bufs=1, space="PSUM") as warm:
                wp = warm.tile([P, P], FP16, name="wp")
                for _ in range(170):
                    nc.tensor.transpose(wp[:], wtile[:], wtile[:])

            ident = const.tile([P, P], FP16, name="ident")
            make_identity(nc, ident[:])

            # ---- resident loads (separate queues) ----
            m_sb = mpool.tile([P, DT, D], FP8, name="m_sb")
            yq_sb = yqpool.tile([P, DT, QSH], FP8, name="yq_sb")
            nc.sync.dma_start(m_sb[:], M8.rearrange("(di p) f -> p di f", p=P))
            nc.sync.dma_start(
                yq_sb[:], Yqt8.rearrange("(di p) f -> p di f", p=P))
            yt_sb = ytpool.tile([P, DT, N], FP8, name="yt_sb")
            nc.gpsimd.dma_start(
                yt_sb[:], Yt8.rearrange("(di p) f -> p di f", p=P))

            # stats: negmax/recip [P, QT], chunk maxes, half sums, rowsum
            st = stat.tile([P, 64], F32, name="st")
            negmax = st[:, 0:QT]
            recip = st[:, QT:2 * QT]
            mx8 = st[:, 8:8 + QT * JC]
            rowsum = st[:, 40:44]
            esum = st[:, 44:44 + 2 * QT]

            ht_sb = htpool.tile([P, DT, QSH], FP8, name="ht_sb")
            s_sb = [
                spool.tile([P, N], FP16, name=f"s_sb{t}", tag=f"s{t}")
                for t in range(QT)
            ]
            pt_sb = [
                ptpool.tile([P, N], FP16, name=f"pt_sb{t}", tag=f"pt{t}")
                for t in range(QT)
            ]
            e_tiles = {}

            def emit_exp_half(t, h, pool):
                e = pool.tile([P, N // 2], FP16, name="e_sb", tag="e")
                e_tiles[(t, h)] = e
                nc.scalar.activation(
                    e[:], s_sb[t][:, h * (N // 2):(h + 1) * (N // 2)],
                    AF.Exp, bias=negmax[:, t:t + 1], scale=1.0,
                    accum_out=esum[:, 2 * t + h:2 * t + h + 1],
                )

            def emit_T_chunk(t, c, pool):
                """Transpose chunk c (512 cols) of E(t) into pt_sb[t]."""
                e = e_tiles[(t, c // 4)]
                off = (c % 4) * 512
                pp = pool.tile([P, 512], FP16, name="pp", tag="pt")
                for k in range(4):
                    nc.tensor.transpose(
                        pp[:, k * P:(k + 1) * P],
                        e[:, off + k * P: off + (k + 1) * P],
                        ident[:],
                    )
                nc.vector.tensor_copy(
                    pt_sb[t][:, c * 512:(c + 1) * 512], pp[:])

            def emit_finish_rowsum(t):
                nc.vector.tensor_reduce(
                    rowsum[:, t:t + 1], esum[:, 2 * t:2 * t + 2],
                    axis=AX.X, op=OP.add,
                )
                nc.vector.reciprocal(recip[:, t:t + 1], rowsum[:, t:t + 1])

            def emit_negmax(t):
                nc.vector.tensor_reduce(
                    negmax[:, t:t + 1], mx8[:, t * JC:(t + 1) * JC],
                    axis=AX.X, op=OP.max, negate=True,
                )

            with tc.tile_pool(name="ps", bufs=2, space="PSUM") as ps:
                # ---- H: Ht[do, q] = sum_di M[di,do].T @ Yqt[di, q] ----
                for dt_ in range(DT):
                    hp = ps.tile([P, QSH], F32, name="hp", tag="s")
                    for s in range(DT // 2):
                        nc.tensor.matmul(
                            hp[:],
                            m_sb[:, 2 * s:2 * s + 2, dt_ * P:(dt_ + 1) * P],
                            yq_sb[:, 2 * s:2 * s + 2, :],
                            start=(s == 0), stop=(s == DT // 2 - 1),
                            perf_mode=DR,
                        )
                    nc.scalar.copy(ht_sb[:, dt_, :], hp[:])

                # ---- S (t-outer), softmax of t-1 interleaved per chunk ----
                for t in range(QT):
                    if t >= 1:
                        emit_negmax(t - 1)
                        emit_exp_half(t - 1, 0, epool)
                    for jc in range(JC):
                        sp = ps.tile([P, 512], F32, name="sp", tag="s")
                        for s in range(DT // 2):
                            nc.tensor.matmul(
                                sp[:],
                                ht_sb[:, 2 * s:2 * s + 2, t * P:(t + 1) * P],
                                yt_sb[:, 2 * s:2 * s + 2,
                                      jc * 512:(jc + 1) * 512],
                                start=(s == 0), stop=(s == DT // 2 - 1),
                                perf_mode=DR,
                            )
                        # alternate eviction engine so neither ACT nor DVE
                        # paces the PE
                        dst = s_sb[t][:, jc * 512:(jc + 1) * 512]
                        if jc % 3 == 2:
                            nc.vector.tensor_copy(dst, sp[:])
                        else:
                            nc.scalar.copy(dst, sp[:])
                        nc.vector.tensor_reduce(
                            mx8[:, t * JC + jc: t * JC + jc + 1], dst,
                            axis=AX.X, op=OP.max,
                        )
                        if t >= 1:
                            if jc == 3:
                                emit_exp_half(t - 1, 1, epool)
                            if jc >= 1:
                                emit_T_chunk(t - 1, jc - 1, ps)
                    if t >= 1:
                        emit_T_chunk(t - 1, JC - 1, ps)
                        emit_finish_rowsum(t - 1)

            # ---- boundary: softmax(3) overlapped with Z for t=0..2 ----
            tl = QT - 1
            with tc.tile_pool(name="psZ", bufs=3, space="PSUM") as psZ:
                zp = {
                    t: psZ.tile([P, D], F32, name=f"zp{t}", tag="z")
                    for t in range(3)
                }

                def z_mms(jt, yz, ts):
                    for t in ts:
                        for dc in range(2):
                            for s in range(2):
                                nc.tensor.matmul(
                                    zp[t][:, dc * 512:(dc + 1) * 512],
                                    pt_sb[t][:, jt * P:(jt + 1) * P],
                                    yz[:, s, dc * 512:dc * 512 + 512],
                                    start=(jt == 0 and s == 0),
                                    stop=(jt == JT - 1 and s == 1),
                                )

                def yz_load(jt):
                    yz = yzpool.tile([P, 2, D], FP16, name="yz")
                    nc.sync.dma_start(yz[:, 0, :], Yh[jt * P:(jt + 1) * P, :])
                    nc.sync.dma_start(yz[:, 1, :], Ym[jt * P:(jt + 1) * P, :])
                    return yz

                yzs = {}
                with tc.tile_pool(name="pp3", bufs=2, space="PSUM") as pp3:
                    emit_negmax(tl)
                    emit_exp_half(tl, 0, epool)
                    for jt in range(4):
                        yzs[jt] = yz_load(jt)
                        z_mms(jt, yzs[jt], (0, 1, 2))
                        if jt == 1:
                            emit_exp_half(tl, 1, epool)
                        for c in (2 * jt, 2 * jt + 1):
                            emit_T_chunk(tl, c, pp3)
                    emit_finish_rowsum(tl)

                # 4th accumulator takes the banks freed by pp3
                with tc.tile_pool(name="psZ2", bufs=1, space="PSUM") as psZ2:
                    zp[3] = psZ2.tile([P, D], F32, name="zp3", tag="z3")
                    for jt in range(4):
                        z_mms(jt, yzs[jt], (3,))
                    for jt in range(4, JT):
                        yz = yz_load(jt)
                        z_mms(jt, yz, (0, 1, 2, 3))
                    for t in range(QT):
                        zo = zopool.tile([P, D], F32, name="zo", tag="zo")
                        nc.scalar.activation(
                            zo[:], zp[t][:], AF.Copy, bias=0.0,
                            scale=recip[:, t:t + 1],
                        )
                        nc.sync.dma_start(Z[t * P:(t + 1) * P, :], zo[:])

    nc.finalize()
    return nc


def _prep_inputs(Y: np.ndarray, W_param: np.ndarray):
    Y = np.ascontiguousarray(Y, dtype=np.float32)
    W = np.ascontiguousarray(W_param, dtype=np.float32)
    M = (W.T @ W).astype(np.float32)
    Yh = Y.astype(np.float16)
    Ym = (Y - Yh.astype(np.float32)).astype(np.float16)
    M8 = M.astype(ml_dtypes.float8_e4m3)
    Yt8 = np.ascontiguousarray(Y.T).astype(ml_dtypes.float8_e4m3)
    in_maps = []
    for c in range(CORES):
        in_maps.append({
            "M8": M8,
            "Yqt8": np.ascontiguousarray(Yt8[:, c * QSH:(c + 1) * QSH]),
            "Yt8": Yt8,
            "Yh": Yh,
            "Ym": Ym,
        })
    return in_maps


def _run(inputs: dict, trace: bool = False):
    Y = np.asarray(inputs["Y"])
    W = np.asarray(inputs["W_param"])
    assert Y.shape == (N, D) and W.shape == (D, D)
    if "nc" not in _CACHED:
        _CACHED["nc"] = _build()
    nc = _CACHED["nc"]
    in_maps = _prep_inputs(Y, W)
    res = run_bass_kernel_spmd(nc, in_maps, list(range(CORES)), trace=trace)
    out = np.concatenate(
        [res.results[c]["Z"] for c in range(CORES)], axis=0
    ).astype(np.float32)
    return out, res


def kernel(Y: np.ndarray, W_param: np.ndarray) -> np.ndarray:
    out, _ = _run({"Y": Y, "W_param": W_param})
    return out
